# revision 6
# baseline (speedup 1.0000x reference)
"""Trainium2 Bass kernel for nn_DomainAdapter (moe_routing).

Reference computation (per sample b with expert e = domain_id[b]):
    h  = gelu(x @ down_W.T + down_b)                  # [S, A]
    h  = where(valid, h @ W_e.T + b_e, h) + emb[e]    # per-sample expert
    y  = LayerNorm(h @ up_W.T + up_b + x) * gamma + beta

Strategy (8 NeuronCores, data-parallel over batch; 4 samples/core):
  - All wire/matmul dtypes are fp16 (PE runs fp16 at bf16 speed = 4x fp32,
    and fp16 halves HBM traffic vs fp32; fp32 PSUM accumulation).
  - Host folding: since up() is linear, up(h3 + expert_b + emb) =
    up(h3) + up_W@(expert_b + emb). Host precomputes the per-sample
    vector u_s = up_b + up_W@(expert_b[e]+emb[e]), adds it into the
    transposed residual input xT (so the PSUM residual trick applies it
    for free) and corrects the gelu bias by -down_W@u_s. This removes
    all bias matmuls and the domain-embedding add from the device.
  - Two passes over the 4096 tokens/core so the ACT engine runs a single
    activation table per pass (Gelu in pass A, Identity in pass B - no
    per-tile table switches):
      pass A: down-proj (xT streamed from DRAM in 8 chunks across 4 DMA
              queues), Gelu+bias on ACT, per-sample expert matmul,
              PSUM->SBUF fp16 evacuation on DVE.
      pass B: up-proj accumulated in PSUM together with the residual
              (x+u_s) via identity matmuls (fused transpose+add), then
              LayerNorm: bn_stats/bn_aggr on DVE, rsqrt via
              fast-inverse-sqrt + 2 Newton steps, normalize on ACT
              (scale/bias form) with fp16 output.
  - ln_gamma/ln_beta applied on host only if non-trivial (they are 1/0).
"""
import numpy as np

from concourse import bacc, tile, mybir
from concourse.bass2jax import (
    _bass_exec_p,
    install_neuronx_cc_hook,
    partition_id_tensor,
)

f32 = mybir.dt.float32
f16 = mybir.dt.float16
i32 = mybir.dt.int32
AF = mybir.ActivationFunctionType
ALU = mybir.AluOpType

B, S, D, A, E = 32, 1024, 1024, 256, 16
N_CORES = 8
NS = B // N_CORES          # samples per core = 4
T = NS * S                 # tokens per core = 4096
CB = 512                   # tokens per pass-A chunk
NB = T // CB               # 8 chunks
NT = T // 128              # 32 pass-B tiles
KD = D // 128              # 8
KA = A // 128              # 2
DH = D // 512              # 2
XQ = T // 512              # x DMA chunks (512 tokens each)
LN_EPS = 1e-5
FISR_C = 0x5F3759DF


def _build_nc(repeat=1):
    nc = bacc.Bacc("TRN2", target_bir_lowering=False, debug=False)

    XT = nc.dram_tensor("xt", [D, T], f16, kind="ExternalInput").ap()
    DWT = nc.dram_tensor("dwt", [D, A], f16, kind="ExternalInput").ap()
    DB2 = nc.dram_tensor("db2", [128, NS * KA], f32, kind="ExternalInput").ap()
    WET = nc.dram_tensor("wet", [NS, A, A], f16, kind="ExternalInput").ap()
    UPT = nc.dram_tensor("upt", [A, D], f16, kind="ExternalInput").ap()
    IDENT = nc.dram_tensor("ident", [128, 128], f16, kind="ExternalInput").ap()
    Y = nc.dram_tensor("y", [T, D], f16, kind="ExternalOutput").ap()

    with tile.TileContext(nc) as tc:
        with (
            tc.tile_pool(name="consts", bufs=1) as cpool,
            tc.tile_pool(name="xtp", bufs=2) as xt_pool,
            tc.tile_pool(name="h3p", bufs=2) as h3_pool,
            tc.tile_pool(name="h1p", bufs=2) as h1_pool,
            tc.tile_pool(name="outp", bufs=3) as out_pool,
            tc.tile_pool(name="stats", bufs=10) as st_pool,
            tc.tile_pool(name="ps", bufs=8, space="PSUM") as ps_pool,
        ):
            dwt = cpool.tile([128, KD, A], f16, tag="dwt")
            nc.scalar.dma_start(dwt[:], DWT.rearrange("(k p) a -> p k a", p=128))
            db2 = cpool.tile([128, NS * KA], f32, tag="db2")
            nc.scalar.dma_start(db2[:], DB2)
            wet = cpool.tile([128, NS, KA, A], f16, tag="wet")
            nc.scalar.dma_start(wet[:], WET.rearrange("s (k p) a -> p s k a", p=128))
            upt = cpool.tile([128, KA, D], f16, tag="upt")
            nc.scalar.dma_start(upt[:], UPT.rearrange("(k p) d -> p k d", p=128))
            ident = cpool.tile([128, 128], f16, tag="ident")
            nc.scalar.dma_start(ident[:], IDENT)

            xq_engines = [nc.sync, nc.gpsimd, nc.scalar, nc.sync,
                          nc.gpsimd, nc.scalar, nc.sync, nc.gpsimd]

            for rep in range(repeat):
                xtg = xt_pool.tile([128, KD, T], f16, tag="xtg",
                                   name=f"xtg_{rep}")
                for q in range(XQ):
                    xq_engines[q % len(xq_engines)].dma_start(
                        xtg[:, :, q * 512:(q + 1) * 512],
                        XT[:, q * 512:(q + 1) * 512]
                        .rearrange("(k p) t -> p k t", p=128),
                    )
                h3 = h3_pool.tile([128, KA, T], f16, tag="h3",
                                  name=f"h3_{rep}")

                # ---- pass A: down-proj + gelu + expert ----
                for b in range(NB):
                    s = b // 2
                    ph1 = [ps_pool.tile([128, CB], f32, tag="ps",
                                        name=f"ph1_{rep}_{b}_{ka}")
                           for ka in range(KA)]
                    for ka in range(KA):
                        for kd in range(KD):
                            nc.tensor.matmul(
                                ph1[ka][:],
                                dwt[:, kd, ka * 128:(ka + 1) * 128],
                                xtg[:, kd, b * CB:(b + 1) * CB],
                                start=(kd == 0), stop=(kd == KD - 1),
                            )
                    h1 = h1_pool.tile([128, KA, CB], f16, tag="h1",
                                      name=f"h1_{rep}_{b}")
                    for ka in range(KA):
                        nc.scalar.activation(
                            h1[:, ka, :], ph1[ka][:], AF.Gelu,
                            bias=db2[:, s * KA + ka:s * KA + ka + 1],
                        )
                    ph2 = [ps_pool.tile([128, CB], f32, tag="ps",
                                        name=f"ph2_{rep}_{b}_{ao}")
                           for ao in range(KA)]
                    for ao in range(KA):
                        for ki in range(KA):
                            nc.tensor.matmul(
                                ph2[ao][:],
                                wet[:, s, ki, ao * 128:(ao + 1) * 128],
                                h1[:, ki, :],
                                start=(ki == 0), stop=(ki == KA - 1),
                            )
                    for ao in range(KA):
                        nc.vector.tensor_copy(
                            h3[:, ao, b * CB:(b + 1) * CB], ph2[ao][:])

                # ---- pass B: up-proj + residual + layernorm ----
                st_engines = [nc.sync, nc.gpsimd]
                for ts in range(NT):
                    px = [ps_pool.tile([128, 512], f32, tag="ps",
                                       name=f"px_{rep}_{ts}_{dh}")
                          for dh in range(DH)]
                    for ki in range(KA):
                        for dh in range(DH):
                            nc.tensor.matmul(
                                px[dh][:],
                                h3[:, ki, ts * 128:(ts + 1) * 128],
                                upt[:, ki, dh * 512:(dh + 1) * 512],
                                start=(ki == 0), stop=False,
                            )
                    for kd in range(KD):
                        dh, j = divmod(kd, 4)
                        nc.tensor.matmul(
                            px[dh][:, j * 128:(j + 1) * 128],
                            xtg[:, kd, ts * 128:(ts + 1) * 128],
                            ident[:],
                            start=False, stop=(j == 3),
                        )

                    st = st_pool.tile([128, 12], f32, tag="st",
                                      name=f"st_{rep}_{ts}")
                    nc.vector.bn_stats(st[:, 0:6], px[0][:])
                    nc.vector.bn_stats(st[:, 6:12], px[1][:])
                    mv = st_pool.tile([128, 2], f32, tag="mv",
                                      name=f"mv_{rep}_{ts}")
                    nc.vector.bn_aggr(mv[:], st[:])
                    varv = mv[:, 1:2]
                    vhn = st_pool.tile([128, 1], f32, tag="vhn",
                                       name=f"vhn_{rep}_{ts}")
                    nc.vector.tensor_scalar(vhn[:], varv, -0.5, -0.5 * LN_EPS,
                                            ALU.mult, ALU.add)
                    yj = st_pool.tile([128, 1], i32, tag="yj",
                                      name=f"yj_{rep}_{ts}")
                    nc.vector.tensor_scalar(yj[:], varv.bitcast(i32), 1, None,
                                            ALU.logical_shift_right)
                    rs = st_pool.tile([128, 1], f32, tag="rs",
                                      name=f"rs_{rep}_{ts}")
                    nc.vector.tensor_scalar(rs[:].bitcast(i32), yj[:], -1,
                                            FISR_C, ALU.mult, ALU.add)
                    q_ = st_pool.tile([128, 1], f32, tag="q",
                                      name=f"q_{rep}_{ts}")
                    tt = st_pool.tile([128, 1], f32, tag="tt",
                                      name=f"tt_{rep}_{ts}")
                    for _ in range(2):
                        nc.vector.tensor_mul(q_[:], rs[:], rs[:])
                        nc.vector.tensor_mul(tt[:], q_[:], vhn[:])
                        nc.vector.tensor_scalar(tt[:], tt[:], 1.5, None,
                                                ALU.add)
                        nc.vector.tensor_mul(rs[:], rs[:], tt[:])
                    nmr = st_pool.tile([128, 1], f32, tag="nmr",
                                       name=f"nmr_{rep}_{ts}")
                    nc.vector.tensor_mul(nmr[:], mv[:, 0:1], rs[:])
                    nc.vector.tensor_scalar_mul(nmr[:], nmr[:], -1.0)

                    if ts % 2 == 0:
                        outt = out_pool.tile([128, 2, D], f16, tag="outt",
                                             name=f"outt_{rep}_{ts // 2}")
                        state_outt = outt
                    else:
                        outt = state_outt
                    for dh in range(DH):
                        nc.scalar.activation(
                            outt[:, ts % 2, dh * 512:(dh + 1) * 512],
                            px[dh][:], AF.Identity, bias=nmr[:], scale=rs[:],
                        )
                    if ts % 2 == 1:
                        pair = ts // 2
                        st_engines[pair % 2].dma_start(
                            Y[pair * 256:(pair + 1) * 256, :]
                            .rearrange("(t p) d -> p t d", p=128),
                            outt[:],
                        )

    nc.compile()
    return nc


class _Runner:
    """jit-once PJRT runner for the SPMD kernel (axon path)."""

    def __init__(self, nc, n_cores):
        import jax
        from jax.sharding import Mesh, PartitionSpec
        from jax.experimental.shard_map import shard_map

        install_neuronx_cc_hook()
        self.nc = nc
        self.n_cores = n_cores
        pname = nc.partition_id_tensor.name if nc.partition_id_tensor else None

        in_names, out_names, out_avals, zero_outs = [], [], [], []
        for alloc in nc.m.functions[0].allocations:
            if not isinstance(alloc, mybir.MemoryLocationSet):
                continue
            name = alloc.memorylocations[0].name
            if alloc.kind == "ExternalInput":
                if name != pname:
                    in_names.append(name)
            elif alloc.kind == "ExternalOutput":
                out_names.append(name)
                shape = tuple(alloc.tensor_shape)
                dtype = mybir.dt.np(alloc.dtype)
                out_avals.append(jax.core.ShapedArray(shape, dtype))
                zero_outs.append(np.zeros(shape, dtype))
        self.in_names = in_names
        self.out_names = out_names
        self.zero_outs = zero_outs
        n_params = len(in_names)
        n_outs = len(out_avals)
        all_in = list(in_names) + list(out_names)
        if pname is not None:
            all_in.append(pname)

        def _body(*args):
            operands = list(args)
            if pname is not None:
                operands.append(partition_id_tensor())
            outs = _bass_exec_p.bind(
                *operands,
                out_avals=tuple(out_avals),
                in_names=tuple(all_in),
                out_names=tuple(out_names),
                lowering_input_output_aliases=(),
                sim_require_finite=True,
                sim_require_nnan=True,
                nc=nc,
            )
            return tuple(outs)

        devices = jax.devices()[:n_cores]
        mesh = Mesh(np.asarray(devices), ("core",))
        in_specs = (PartitionSpec("core"),) * (n_params + n_outs)
        out_specs = (PartitionSpec("core"),) * n_outs
        self._fn = jax.jit(
            shard_map(_body, mesh=mesh, in_specs=in_specs,
                      out_specs=out_specs, check_rep=False),
            keep_unused=True,
        )

    def run_concat(self, concat_map):
        """concat_map: name -> np array with per-core blocks stacked on axis 0."""
        args = [concat_map[k] for k in self.in_names]
        zeros = [np.concatenate([z] * self.n_cores, axis=0) for z in self.zero_outs]
        outs = self._fn(*args, *zeros)
        return {name: np.asarray(o) for name, o in zip(self.out_names, outs)}


_RUNNER_CACHE = {}


def _get_runner(repeat=1):
    key = repeat
    if key not in _RUNNER_CACHE:
        _RUNNER_CACHE[key] = _Runner(_build_nc(repeat=repeat), N_CORES)
    return _RUNNER_CACHE[key]


def _prep_concat(hidden_states, domain_id, down_W, down_b, up_W, up_b,
                 expert_W, expert_b, domain_emb):
    hs = np.asarray(hidden_states, dtype=np.float32)
    dom = np.asarray(domain_id)
    valid = (dom >= 0) & (dom < E)
    idx = np.clip(dom, 0, E - 1).astype(np.int64)

    down_W = np.asarray(down_W, dtype=np.float64)
    down_b = np.asarray(down_b, dtype=np.float64)
    up_W = np.asarray(up_W, dtype=np.float64)
    up_b = np.asarray(up_b, dtype=np.float64)
    expert_W = np.asarray(expert_W, dtype=np.float32)
    expert_b = np.asarray(expert_b, dtype=np.float64)
    domain_emb = np.asarray(domain_emb, dtype=np.float64)

    # per-sample folded bias: u_s = up_b + up_W @ (expert_b[e] + emb[e])
    # (expert_b only when the id is valid); corrected gelu bias
    # db'_s = down_b - down_W @ u_s; expert weights gathered+transposed
    # (invalid ids -> identity so the on-device math is uniform).
    beme = domain_emb[idx] + np.where(valid[:, None], expert_b[idx], 0.0)
    u = up_b[None, :] + beme @ up_W.T                  # [B, D] f64
    dbp = down_b[None, :] - u @ down_W.T               # [B, A] f64
    wet = np.where(valid[:, None, None],
                   expert_W[idx].transpose(0, 2, 1),
                   np.eye(A, dtype=np.float32)[None]).astype(np.float16)

    # residual input: xT per core with u_s folded in, fp16, [D, T]
    x_aug = hs + u.astype(np.float32)[:, None, :]      # [B, S, D]
    xt = np.ascontiguousarray(
        x_aug.reshape(N_CORES, T, D).transpose(0, 2, 1)).astype(np.float16)

    db2 = np.ascontiguousarray(
        dbp.astype(np.float32).reshape(N_CORES, NS, KA, 128)
        .transpose(0, 3, 1, 2).reshape(N_CORES, 128, NS * KA))

    dwT = np.ascontiguousarray(down_W.T).astype(np.float16)
    upT = np.ascontiguousarray(up_W.T).astype(np.float16)
    ident = np.eye(128, dtype=np.float16)

    concat = {
        "xt": xt.reshape(N_CORES * D, T),
        "dwt": np.concatenate([dwT] * N_CORES, axis=0),
        "db2": db2.reshape(N_CORES * 128, NS * KA),
        "wet": wet.reshape(N_CORES * NS, A, A),
        "upt": np.concatenate([upT] * N_CORES, axis=0),
        "ident": np.concatenate([ident] * N_CORES, axis=0),
    }
    return concat


def kernel(hidden_states, domain_id, down_W, down_b, up_W, up_b,
           expert_W, expert_b, domain_emb, ln_gamma, ln_beta):
    concat = _prep_concat(hidden_states, domain_id, down_W, down_b,
                          up_W, up_b, expert_W, expert_b, domain_emb)
    runner = _get_runner()
    outs = runner.run_concat(concat)
    y = outs["y"].astype(np.float32).reshape(B, S, D)

    g = np.asarray(ln_gamma, dtype=np.float32)
    bta = np.asarray(ln_beta, dtype=np.float32)
    if not (np.all(g == 1.0) and np.all(bta == 0.0)):
        y = y * g + bta
    return y
